# revision 1
# baseline (speedup 1.0000x reference)
"""ConvAutoencoder on 8 Trainium2 NeuronCores.

Sharding: 8 shards = batch(2) x d-slab(4). Each core receives a
zero-padded d-slab of 40 planes (32 output planes + 4-plane halo on
each side; halo>=3 is what the conv3/pool/conv3 receptive field needs,
4 keeps the 2x2x2 pools aligned). Each core runs the full
encoder/decoder pipeline on its slab independently -- no collectives --
and the host trims the halo and reassembles the full output.
"""

import numpy as np

B, G = 2, 128
NSHARD = 8
DOUT = G // 4          # 32 output planes per shard
HALO = 4
DSLAB = DOUT + 2 * HALO  # 40


def _build_slabs(x, occ):
    xs = np.zeros((NSHARD, DSLAB, G, G, 1), np.float32)
    # pad occ with 1 -> mask (occ==0) is 0 outside the volume
    os_ = np.ones((NSHARD, DSLAB, G, G), np.int32)
    for c in range(NSHARD):
        b, q = divmod(c, 4)
        dlo, dhi = 32 * q - HALO, 32 * q + DOUT + HALO
        slo, shi = max(dlo, 0), min(dhi, G)
        xs[c, slo - dlo:shi - dlo] = x[b, slo:shi]
        os_[c, slo - dlo:shi - dlo] = occ[b, slo:shi]
    return xs, os_


def _pipeline(x, occ, W1, W2, Wt1, Wt2):
    import jax
    import jax.numpy as jnp
    from jax import lax

    DN = ('NDHWC', 'DHWIO', 'NDHWC')

    def conv(h, W):
        return lax.conv_general_dilated(h, W, (1, 1, 1), 'SAME',
                                        dimension_numbers=DN)

    def tconv(h, W):
        return lax.conv_transpose(h, W, (2, 2, 2), 'VALID',
                                  dimension_numbers=DN)

    def pool(h):
        n, d1, d2, d3, c = h.shape
        return h.reshape(n, d1 // 2, 2, d2 // 2, 2, d3 // 2, 2, c).max(
            axis=(2, 4, 6))

    def pool_m(m):
        n, d1, d2, d3 = m.shape
        return m.reshape(n, d1 // 2, 2, d2 // 2, 2, d3 // 2, 2).max(
            axis=(2, 4, 6))

    def up_m(m):
        return jnp.repeat(jnp.repeat(jnp.repeat(m, 2, 1), 2, 2), 2, 3)

    m0 = (occ == 0).astype(x.dtype)
    h = jax.nn.relu(conv(x * m0[..., None], W1)) * m0[..., None]
    h = pool(h); m1 = pool_m(m0)
    h = jax.nn.relu(conv(h, W2)) * m1[..., None]
    h = pool(h); m2 = pool_m(m1)
    m3 = up_m(m2)
    h = jax.nn.relu(tconv(h, Wt1)) * m3[..., None]
    m4 = up_m(m3)
    return jax.nn.sigmoid(tconv(h, Wt2)) * m4[..., None]


_CACHE = {}


def _get_fn():
    if 'fn' not in _CACHE:
        import jax
        devs = jax.devices()[:NSHARD]

        def per_shard(x, occ, W1, W2, Wt1, Wt2):
            return _pipeline(x[None], occ[None], W1, W2, Wt1, Wt2)[0]

        _CACHE['fn'] = jax.pmap(
            per_shard, devices=devs,
            in_axes=(0, 0, None, None, None, None))
    return _CACHE['fn']


def kernel(x, W1, W2, Wt1, Wt2, occ):
    x = np.asarray(x, np.float32)
    occ = np.asarray(occ, np.int32)
    xs, os_ = _build_slabs(x, occ)
    fn = _get_fn()
    out_slabs = np.asarray(fn(xs, os_,
                              np.asarray(W1, np.float32),
                              np.asarray(W2, np.float32),
                              np.asarray(Wt1, np.float32),
                              np.asarray(Wt2, np.float32)))
    out = np.empty((B, G, G, G, 1), np.float32)
    for c in range(NSHARD):
        b, q = divmod(c, 4)
        out[b, 32 * q:32 * q + DOUT] = out_slabs[c, HALO:HALO + DOUT]
    return out

